# revision 39
# baseline (speedup 1.0000x reference)
"""Trainium2 8-core tensor-parallel sparse-attention kernel (Bass/Tile).

Reference (SQ=2048, B=1, H=2048, NH=16, HD=128):
    x = hidden[:,0,:] @ svd_token
    w = qkv_w @ svd_token;  mixed = x @ w.T + qkv_b
    per head h: q,k rotated by svd_qk[h], v by svd_vlin[h]
    scores = qr @ kr.T / sqrt(128) causal-masked, softmax
    ctx = probs @ vr;  tsr[h] = svd_vlin[h].T @ dense_w[h]
    out = ctx @ tsr + dense_b

Key identity: mixed = x @ (qkv_w @ st).T = (x @ st.T) @ qkv_w.T = y @ qkv_w.T
 -> compute y seq-sharded (2.15 GF/core) instead of w head-sharded (6.4).

Per-core pipeline (TP over heads, 2 heads/core):
  warmup AG first (CC stream init ~55us)
  All bulk HBM loads stream on the sync queue in consumption order
  (ring FIFO = program order): hT tiles + stok rows batched in pairs
  (pass1), stokT half-rows (pass2 per j2-half), qwT, dw.
  pass2 per j2-half -> AG_y per half triggers early, overlaps compute
  B2:    mixed^T = qkv_w-shard @ y^T, rank-pairs N=512, fused
         (+bias+half-merge) epilogue via scalar_tensor_tensor
  C:     rotations for both heads up front (own PSUM pool, PSUM->SBUF
         copies split across scalar/vector); per head, two rb-chains
         interleaved for PE density; causal mask preloaded into PSUM
         by an identity matmul (no cross-engine mask hop); raw exp on
         [128,1024] paired tiles; P@V + ones row-sum; normalize via
         partition_broadcast + reciprocal_approx_fast
  A2A:   per head -- A2A(h0) hides under h1's attention
  E:     out = ctx_myblock @ tsr + dense_b, split into h0/h1 halves so
         the h0 half of the contraction overlaps A2A(h1)
Host only shards inputs / concatenates the 8 output row-blocks.
"""
import math

import ml_dtypes
import numpy as np

import concourse.bass as bass
import concourse.mybir as mybir
import concourse.bacc as bacc
import concourse.tile as tile
from concourse import bass_utils

N_CORES = 8
SQ = 2048
H = 2048
NH = 16
HD = 128
HPC = NH // N_CORES          # heads per core = 2
QKVR = HPC * 3 * HD          # qkv rows per core = 768
SEQB = SQ // N_CORES         # seq block per core = 256
KT = H // 128                # 128-tiles over hidden = 16
MT = QKVR // 128             # qkv row tiles = 6
F32 = mybir.dt.float32
F32R = mybir.dt.float32r
BF16 = mybir.dt.bfloat16
FP16 = mybir.dt.float16
SCALE = 1.0 / math.sqrt(HD)


def r(ap):
    return ap.bitcast(F32R)


def build(causal=True):
    nc = bacc.Bacc("TRN2", target_bir_lowering=False, debug=False,
                   num_devices=N_CORES)

    hT = nc.dram_tensor("hT", [H, SEQB], FP16, kind="ExternalInput")
    qwT = nc.dram_tensor("qwT", [H, QKVR], FP16, kind="ExternalInput")
    qbT = nc.dram_tensor("qbT", [128, MT], F32, kind="ExternalInput")
    stok = nc.dram_tensor("stok", [H, H], FP16, kind="ExternalInput")
    stokT = nc.dram_tensor("stokT", [H, H], FP16, kind="ExternalInput")
    sqk = nc.dram_tensor("sqk", [HPC, HD, HD], FP16, kind="ExternalInput")
    svl = nc.dram_tensor("svl", [HPC, HD, HD], FP16, kind="ExternalInput")
    dw = nc.dram_tensor("dw", [HPC, HD, H], FP16, kind="ExternalInput")
    dbB = nc.dram_tensor("dbB", [1, H], F32, kind="ExternalInput")
    out = nc.dram_tensor("out", [SEQB, H], F32, kind="ExternalOutput")

    ones_dram = nc.inline_tensor(np.ones((128, 128), np.float32), name="ones_c")
    onesb_dram = nc.inline_tensor(np.ones((128, 128), ml_dtypes.bfloat16),
                                  name="onesb_c")
    idh_dram = nc.inline_tensor(np.eye(128, dtype=np.float16), name="idh_c")
    # additive causal mask band (-30000 above diagonal), preloaded
    # into PSUM via an identity matmul so masking never leaves the PE
    tbh_np = np.where(
        np.arange(128)[:, None] > np.arange(896)[None, :] - 384, -30000.0, 0.0
    ).astype(np.float16)
    tbh_dram = nc.inline_tensor(tbh_np, name="tbh_c")

    rg = [list(range(N_CORES))]

    with tile.TileContext(nc) as tc:
        with (
            nc.allow_low_precision(reason="f32r/bf16 for full-rate PE"),
            tc.tile_pool(name="pers", bufs=1) as pers,
            tc.tile_pool(name="dram", bufs=1, space="DRAM") as dram,
        ):
            # ---- warmup collective ASAP (CC stream init ~55us) ----
            warm_in = dram.tile([128, 128], F32)
            warm_out = dram.tile([N_CORES * 128, 128], F32,
                                 addr_space="Shared")
            nc.sync.dma_start(warm_in[:], ones_dram[:])
            nc.gpsimd.collective_compute(
                "AllGather", mybir.AluOpType.bypass, replica_groups=rg,
                ins=[warm_in[:].opt()], outs=[warm_out[:].opt()])

            # ---- persistent constants ----
            ones_sb = pers.tile([128, 128], F32)
            onesb_sb = pers.tile([128, 128], BF16)
            tbh_sb = pers.tile([128, 896], FP16)
            idb_sb = pers.tile([128, 128], FP16)
            nc.gpsimd.dma_start(idb_sb[:], idh_dram[:])
            nc.sync.dma_start(r(ones_sb[:]), r(ones_dram[:]))
            nc.gpsimd.dma_start(onesb_sb[:], onesb_dram[:])
            nc.gpsimd.dma_start(tbh_sb[:], tbh_dram[:])
            qb_sb = pers.tile([128, MT], F32)
            nc.sync.dma_start(qb_sb[:], qbT[:])
            sqk_sb = pers.tile([128, HPC * HD], FP16)
            svl_sb = pers.tile([128, HPC * HD], FP16)
            for hl in range(HPC):
                nc.gpsimd.dma_start(sqk_sb[:, hl * HD:(hl + 1) * HD], sqk[hl])
                nc.gpsimd.dma_start(svl_sb[:, hl * HD:(hl + 1) * HD], svl[hl])
            db_sb = pers.tile([1, H], F32)
            nc.sync.dma_start(r(db_sb[:]), r(dbB[:]))

            y_ins = [dram.tile([H // 2, SEQB], FP16, name=f"y_in{q}")
                     for q in range(2)]
            y_gs = [dram.tile([N_CORES * H // 2, SEQB], FP16,
                              addr_space="Shared", name=f"y_g{q}")
                    for q in range(2)]
            tsr_in = dram.tile([HPC * HD, H], FP16)
            tsr_g = dram.tile([NH * HD, H], FP16, addr_space="Shared")

            # ---- pass1/pass2 (sync queue = consumption-ordered stream) ----
            dw_sb = pers.tile([128, HPC * H], FP16)
            with (
                tc.tile_pool(name="sA", bufs=2) as sA,
                tc.tile_pool(name="pA", bufs=8, space="PSUM") as pA,
            ):
                hT_sb = sA.tile([128, KT * SEQB], FP16, tag="hTt", bufs=1)
                # pass1: x[s, j] = sum_k hT[k, s].T @ stok[k, j]
                # (hT tile k and stok row k interleaved in ring order)
                xps = [pA.tile([128, 512], F32, tag="acc", name=f"xps{i}",
                               bufs=8) for i in range(8)]
                stok_v = stok.rearrange("(m p) j -> m p j", p=128)
                for k2 in range(KT // 2):
                    nc.sync.dma_start(
                        hT_sb[:].rearrange(
                            "p (k s) -> p k s", k=KT)[:, k2 * 2:(k2 + 1) * 2],
                        hT.rearrange("(k p) s -> p k s",
                                     p=128)[:, k2 * 2:(k2 + 1) * 2])
                    srow = sA.tile([128, 2 * H], FP16, tag="srow", bufs=4)
                    nc.sync.dma_start(
                        srow[:].rearrange("p (m j) -> p m j", m=2),
                        stok_v[k2 * 2:(k2 + 1) * 2].rearrange(
                            "m p j -> p m j"))
                    for ki in range(2):
                        k = k2 * 2 + ki
                        for st in range(2):
                            for jc in range(4):
                                nc.tensor.matmul(
                                    xps[st * 4 + jc][:],
                                    hT_sb[:, k * SEQB + st * 128:
                                          k * SEQB + (st + 1) * 128],
                                    srow[:, ki * H + jc * 512:
                                         ki * H + (jc + 1) * 512],
                                    start=(k == 0), stop=(k == KT - 1))
                x_sb = sA.tile([128, 2 * H], FP16, tag="xsb", bufs=1)
                for i in range(8):
                    if i % 2:
                        nc.vector.tensor_copy(
                            x_sb[:, i * 512:(i + 1) * 512], xps[i][:])
                    else:
                        nc.scalar.copy(
                            x_sb[:, i * 512:(i + 1) * 512], xps[i][:])
                # transpose x -> xT (m on partitions)
                xT_sb = sA.tile([128, KT * SEQB], FP16, tag="xTt", bufs=1)
                for m in range(KT):
                    tp2 = pA.tile([128, 256], FP16, tag="acc",
                                  name=f"tpx{m}", bufs=8)
                    for st in range(2):
                        nc.tensor.transpose(
                            tp2[:, st * 128:(st + 1) * 128],
                            x_sb[:, st * H + m * 128: st * H + (m + 1) * 128],
                            idb_sb[:])
                    if m % 2:
                        nc.vector.tensor_copy(
                            xT_sb[:, m * SEQB:(m + 1) * SEQB], tp2[:])
                    else:
                        nc.scalar.copy(
                            xT_sb[:, m * SEQB:(m + 1) * SEQB], tp2[:])
                # pass2 per j2-half: y[s, j2] = sum_m xT[m, s].T @ stokT[m, j2]
                y_sb = sA.tile([128, 2 * H], FP16, tag="xsb", bufs=1,
                               name="y_sb")
                for hf in range(2):
                    yps = [pA.tile([128, 512], F32, tag="acc",
                                   name=f"yps{hf}_{i}", bufs=8)
                           for i in range(4)]
                    stokT_v = stokT.rearrange("(m p) j -> m p j", p=128)
                    for m2 in range(KT // 2):
                        srow2 = sA.tile([128, 2048], FP16, tag="srow2",
                                        bufs=4)
                        nc.sync.dma_start(
                            srow2[:].rearrange("p (m j) -> p m j", m=2),
                            stokT_v[m2 * 2:(m2 + 1) * 2, :,
                                    hf * 1024:(hf + 1) * 1024]
                            .rearrange("m p j -> p m j"))
                        for mi in range(2):
                            m = m2 * 2 + mi
                            for st in range(2):
                                for jc in range(2):
                                    nc.tensor.matmul(
                                        yps[st * 2 + jc][:],
                                        xT_sb[:, m * SEQB + st * 128:
                                              m * SEQB + (st + 1) * 128],
                                        srow2[:, mi * 1024 + jc * 512:
                                              mi * 1024 + (jc + 1) * 512],
                                        start=(m == 0), stop=(m == KT - 1))
                    for st in range(2):
                        for jc in range(2):
                            if (st + jc) % 2:
                                nc.vector.tensor_copy(
                                    y_sb[:, st * H + hf * 1024 + jc * 512:
                                         st * H + hf * 1024 + (jc + 1) * 512],
                                    yps[st * 2 + jc][:])
                            else:
                                nc.scalar.copy(
                                    y_sb[:, st * H + hf * 1024 + jc * 512:
                                         st * H + hf * 1024 + (jc + 1) * 512],
                                    yps[st * 2 + jc][:])
                    # transpose this half's j2 tiles, stage 4-at-a-time, AG
                    for jq4 in range(2):
                        ystg = sA.tile([128, 4 * SEQB], FP16, tag="ystg",
                                       bufs=2)
                        for jj in range(4):
                            j2 = hf * 8 + jq4 * 4 + jj
                            tp3 = pA.tile([128, 256], FP16, tag="acc",
                                          name=f"tpy{j2}", bufs=8)
                            for st in range(2):
                                nc.tensor.transpose(
                                    tp3[:, st * 128:(st + 1) * 128],
                                    y_sb[:, st * H + j2 * 128:
                                         st * H + (j2 + 1) * 128],
                                    idb_sb[:])
                            if jj % 2:
                                nc.vector.tensor_copy(
                                    ystg[:, jj * SEQB:(jj + 1) * SEQB],
                                    tp3[:])
                            else:
                                nc.scalar.copy(
                                    ystg[:, jj * SEQB:(jj + 1) * SEQB],
                                    tp3[:])
                        nc.sync.dma_start(
                            y_ins[hf].rearrange(
                                "(j p) s -> p j s", p=128)[:, jq4 * 4:
                                                           (jq4 + 1) * 4],
                            ystg[:].rearrange("p (j s) -> p j s", j=4))
                    nc.gpsimd.collective_compute(
                        "AllGather", mybir.AluOpType.bypass,
                        replica_groups=rg,
                        ins=[y_ins[hf][:].opt()],
                        outs=[y_gs[hf][:].opt()])
                # bulk loads for later stages, ring-ordered after streams
                qwT_sb = pers.tile([128, KT * QKVR], FP16)
                for k in range(KT):
                    nc.sync.dma_start(
                        qwT_sb[:, k * QKVR:(k + 1) * QKVR],
                        qwT[k * 128:(k + 1) * 128, :])
                for hl in range(HPC):
                    nc.sync.dma_start(dw_sb[:, hl * H:(hl + 1) * H], dw[hl])

            # ---- tsr shard + AG_tsr + dense_b broadcast ----
            bb_sb = pers.tile([128, H], FP16)
            with (
                tc.tile_pool(name="s0", bufs=2) as s0,
                tc.tile_pool(name="p0", bufs=2, space="PSUM") as p0,
            ):
                for hl in range(HPC):
                    tsr_sb = s0.tile([128, H], FP16, tag="tsr")
                    for n in range(4):
                        tp = p0.tile([128, 512], F32, tag="t0p")
                        nc.tensor.matmul(
                            tp[:], svl_sb[:, hl * HD:(hl + 1) * HD],
                            dw_sb[:, hl * H + n * 512: hl * H + (n + 1) * 512],
                            start=True, stop=True)
                        nc.vector.tensor_copy(
                            tsr_sb[:, n * 512:(n + 1) * 512], tp[:])
                    nc.sync.dma_start(tsr_in[hl * HD:(hl + 1) * HD, :],
                                      tsr_sb[:])
                for n in range(4):
                    bp = p0.tile([128, 512], F32, tag="t0p")
                    nc.tensor.matmul(bp[:], r(ones_sb[0:1, :]),
                                     r(db_sb[:, n * 512:(n + 1) * 512]),
                                     start=True, stop=True)
                    nc.vector.tensor_copy(bb_sb[:, n * 512:(n + 1) * 512],
                                          bp[:])
            nc.gpsimd.collective_compute(
                "AllGather", mybir.AluOpType.bypass, replica_groups=rg,
                ins=[tsr_in[:].opt()], outs=[tsr_g[:].opt()])

            # ---- B2: mixed^T = qw_shard @ y^T, rank-pairs (N=512) ----
            mid = tc.alloc_tile_pool(name="mid", bufs=1)
            mixT = mid.tile([128, MT * SQ], FP16, name="mixT")
            tsrb_sb = mid.tile([128, KT * H], FP16, name="tsrb_sb")
            with (
                tc.tile_pool(name="sB", bufs=2) as sB,
                tc.tile_pool(name="pB", bufs=6, space="PSUM") as pB,
            ):
                half1 = sB.tile([128, 4 * MT * 512], FP16, tag="half1",
                                bufs=1)
                ygv = [y_gs[q].rearrange("(r k p) s -> r k p s",
                                         r=N_CORES, p=128) for q in range(2)]
                for rp in range(N_CORES // 2):
                    mps = [pB.tile([128, 512], F32, tag="mp",
                                   name=f"mpa{rp}_{i}") for i in range(MT)]
                    for kb in range(2):
                        yg_sb = sB.tile([128, 4 * 512], FP16, tag="yg",
                                        bufs=4)
                        for h in range(2):
                            nc.sync.dma_start(
                                yg_sb[:].rearrange("p (k h s) -> h p k s",
                                                   k=4, h=2)[h],
                                ygv[0][rp * 2 + h, kb * 4:(kb + 1) * 4]
                                .rearrange("k p s -> p k s"))
                        for k4 in range(4):
                            k = kb * 4 + k4
                            for mt in range(MT):
                                nc.tensor.matmul(
                                    mps[mt][:],
                                    qwT_sb[:, k * QKVR + mt * 128:
                                           k * QKVR + (mt + 1) * 128],
                                    yg_sb[:, k4 * 512:(k4 + 1) * 512],
                                    start=(k == 0), stop=(k == KT // 2 - 1))
                    for mt in range(MT):
                        nc.vector.tensor_copy(
                            half1[:, (rp * MT + mt) * 512:
                                  (rp * MT + mt + 1) * 512], mps[mt][:])
                for rp in range(N_CORES // 2):
                    mps2 = [pB.tile([128, 512], F32, tag="mp",
                                    name=f"mpb{rp}_{i}") for i in range(MT)]
                    for kb in range(2):
                        yg_sb = sB.tile([128, 4 * 512], FP16, tag="yg",
                                        bufs=4)
                        for h in range(2):
                            nc.sync.dma_start(
                                yg_sb[:].rearrange("p (k h s) -> h p k s",
                                                   k=4, h=2)[h],
                                ygv[1][rp * 2 + h, kb * 4:(kb + 1) * 4]
                                .rearrange("k p s -> p k s"))
                        for k4 in range(4):
                            k = KT // 2 + kb * 4 + k4
                            for mt in range(MT):
                                nc.tensor.matmul(
                                    mps2[mt][:],
                                    qwT_sb[:, k * QKVR + mt * 128:
                                           k * QKVR + (mt + 1) * 128],
                                    yg_sb[:, k4 * 512:(k4 + 1) * 512],
                                    start=(k == KT // 2), stop=(k == KT - 1))
                    for mt in range(MT):
                        nc.vector.scalar_tensor_tensor(
                            mixT[:, mt * SQ + rp * 512:
                                 mt * SQ + (rp + 1) * 512],
                            mps2[mt][:], qb_sb[:, mt:mt + 1],
                            half1[:, (rp * MT + mt) * 512:
                                  (rp * MT + mt + 1) * 512],
                            mybir.AluOpType.add, mybir.AluOpType.add)

            # ---- stage C: rotations + attention per head ----
            ctx_in = dram.tile([N_CORES, HPC * HD, SEQB], FP16, name="ctxin")
            ctx_a = dram.tile([N_CORES, HPC * HD, SEQB], FP16, name="ctxa")
            # prefetch tsr during stage C on the sync queue (idle there);
            # keeping it off gpsimd stops its AG_tsr wait from blocking
            # the normalize partition_broadcasts during attention
            for kt in range(KT):
                nc.sync.dma_start(
                    tsrb_sb[:, kt * H:(kt + 1) * H],
                    tsr_g[kt * 128:(kt + 1) * 128, :])
            with (
                tc.tile_pool(name="sC", bufs=1) as sC,
                tc.tile_pool(name="sD", bufs=2) as sD,
            ):
                qrotTs, krotTs, vrots, ctxTs = [], [], [], []
                with tc.tile_pool(name="pR", bufs=2, space="PSUM") as pR:
                    for hl in range(HPC):
                        qrow, krow, vrow = hl * 3, hl * 3 + 1, hl * 3 + 2
                        qrotT = sC.tile([128, SQ], FP16, tag="qrot", bufs=2,
                                        name=f"qrotT{hl}")
                        krotT = sC.tile([128, SQ], FP16, tag="krot", bufs=2,
                                        name=f"krotT{hl}")
                        vrot = sC.tile([128, SQ], BF16, tag="vrot", bufs=2,
                                       name=f"vrot{hl}")
                        for sc in range(4):
                            rp1 = pR.tile([128, 512], F32, tag="rotp")
                            nc.tensor.matmul(
                                rp1[:], sqk_sb[:, hl * HD:(hl + 1) * HD],
                                mixT[:, qrow * SQ + sc * 512:
                                     qrow * SQ + (sc + 1) * 512],
                                start=True, stop=True)
                            nc.scalar.copy(
                                qrotT[:, sc * 512:(sc + 1) * 512], rp1[:])
                            rp2 = pR.tile([128, 512], F32, tag="rotp")
                            nc.tensor.matmul(
                                rp2[:], sqk_sb[:, hl * HD:(hl + 1) * HD],
                                mixT[:, krow * SQ + sc * 512:
                                     krow * SQ + (sc + 1) * 512],
                                start=True, stop=True)
                            nc.vector.tensor_copy(
                                krotT[:, sc * 512:(sc + 1) * 512], rp2[:])
                        for st in range(KT):
                            vp = pR.tile([128, 128], F32, tag="rotp")
                            nc.tensor.matmul(
                                vp[:],
                                mixT[:, vrow * SQ + st * 128:
                                     vrow * SQ + (st + 1) * 128],
                                svl_sb[:, hl * HD:(hl + 1) * HD],
                                start=True, stop=True)
                            if st % 2:
                                nc.vector.tensor_copy(
                                    vrot[:, st * 128:(st + 1) * 128], vp[:])
                            else:
                                nc.scalar.copy(
                                    vrot[:, st * 128:(st + 1) * 128], vp[:])
                        ctxT_sb = sC.tile([128, SQ], FP16, tag="ctxT",
                                          bufs=2, name=f"ctxT{hl}")
                        qrotTs.append(qrotT); krotTs.append(krotT)
                        vrots.append(vrot); ctxTs.append(ctxT_sb)

                with tc.tile_pool(name="pC", bufs=2, space="PSUM") as pC:
                    for rb in range(4):
                        ncb = 4 * (rb + 1) if causal else KT
                        ctps = [pC.tile([128, 512], F32, tag="ctp",
                                        name=f"ctp{hl}_{rb}")
                                for hl in range(HPC)]
                        lps = [pC.tile([1, 512], F32, tag="lp", bufs=2,
                                       name=f"lp{hl}_{rb}")
                               for hl in range(HPC)]
                        # interleave the two heads: while one head's chain
                        # waits on exp, the other's matmuls fill the PE
                        for cp in range(ncb // 2):
                            for hl in range(HPC):
                                qrotT, krotT = qrotTs[hl], krotTs[hl]
                                vrot = vrots[hl]
                                sp = pC.tile([128, 1024], F32, tag="sp",
                                             name=f"sp{hl}_{rb}_{cp}")
                                pT = sD.tile([128, 1024], BF16, tag="pT",
                                             bufs=6,
                                             name=f"pT{hl}_{rb}_{cp}")
                                for ch in range(2):
                                    cb = cp * 2 + ch
                                    sph = sp[:, ch * 512:(ch + 1) * 512]
                                    masked = causal and cb >= 4 * rb
                                    if masked:
                                        o = 384 - (cb * 128 - rb * 512)
                                        nc.tensor.matmul(
                                            sph, idb_sb[:],
                                            tbh_sb[:, o:o + 512],
                                            start=True, stop=False)
                                    nc.tensor.matmul(
                                        sph,
                                        krotT[:, cb * 128:(cb + 1) * 128],
                                        qrotT[:, rb * 512:(rb + 1) * 512],
                                        start=not masked, stop=True)
                                nc.scalar.activation(
                                    pT[:], sp[:],
                                    mybir.ActivationFunctionType.Exp)
                                for ch in range(2):
                                    cb = cp * 2 + ch
                                    pTh = pT[:, ch * 512:(ch + 1) * 512]
                                    nc.tensor.matmul(
                                        ctps[hl][:],
                                        vrot[:, cb * 128:(cb + 1) * 128],
                                        pTh,
                                        start=(cb == 0),
                                        stop=(cb == ncb - 1))
                                    nc.tensor.matmul(
                                        lps[hl][:], onesb_sb[:, 0:1], pTh,
                                        start=(cb == 0),
                                        stop=(cb == ncb - 1))
                        for hl in range(HPC):
                            ctxT_sb = ctxTs[hl]
                            # normalize: lp -> SBUF, partition-broadcast,
                            # fast reciprocal (128 lanes), mult
                            lsb = sD.tile([1, 512], F32, tag="lsb",
                                          name=f"lsb{hl}_{rb}")
                            nc.scalar.copy(lsb[:], lps[hl][:])
                            lball = sD.tile([128, 512], F32, tag="lball",
                                            name=f"lball{hl}_{rb}")
                            nc.gpsimd.partition_broadcast(lball[:], lsb[:])
                            linvb = sD.tile([128, 512], F32, tag="lb",
                                            name=f"linvb{hl}_{rb}")
                            nc.vector.reciprocal_approx_fast(linvb[:],
                                                             lball[:])
                            nc.vector.tensor_tensor(
                                ctxT_sb[:, rb * 512:(rb + 1) * 512],
                                ctps[hl][:],
                                linvb[:], mybir.AluOpType.mult)
                            # stage ctx columns (dest cores 2rb, 2rb+1)
                            for half in range(2):
                                b = 2 * rb + half
                                nc.sync.dma_start(
                                    ctx_in[b, hl * HD:(hl + 1) * HD, :],
                                    ctxT_sb[:, b * SEQB:(b + 1) * SEQB])
                nc.gpsimd.collective_compute(
                    "AllToAll", mybir.AluOpType.bypass, replica_groups=rg,
                    ins=[ctx_in[:].opt()], outs=[ctx_a[:].opt()])

            # ---- stage E: out = ctx_myblock @ tsr + dense_b ----
            with (
                tc.tile_pool(name="sE", bufs=2) as sE,
                tc.tile_pool(name="pE", bufs=4, space="PSUM") as pE,
            ):
                ctxa_sb = sE.tile([128, KT * SEQB], FP16, tag="ctxa", bufs=1)
                for b in range(N_CORES):
                    nc.sync.dma_start(
                        ctxa_sb[:].rearrange("p (b h s) -> b p h s",
                                             b=N_CORES, h=HPC)[b],
                        ctx_a[b].rearrange("(h p) s -> p h s", p=128))
                for mt in range(2):
                    ops = [pE.tile([128, 512], F32, tag="op",
                                   name=f"op{mt}_{n}") for n in range(4)]
                    for kt in range(KT):
                        for n in range(4):
                            nc.tensor.matmul(
                                ops[n][:],
                                ctxa_sb[:, kt * SEQB + mt * 128:
                                        kt * SEQB + (mt + 1) * 128],
                                tsrb_sb[:, kt * H + n * 512:
                                        kt * H + (n + 1) * 512],
                                start=(kt == 0), stop=(kt == KT - 1))
                    for n in range(4):
                        os_ = sE.tile([128, 512], F32, tag="os")
                        nc.vector.tensor_tensor(
                            os_[:], ops[n][:],
                            bb_sb[:, n * 512:(n + 1) * 512],
                            mybir.AluOpType.add)
                        nc.sync.dma_start(
                            out[mt * 128:(mt + 1) * 128,
                                n * 512:(n + 1) * 512],
                            os_[:])
            mid.release()
    nc.compile()
    return nc


_CAUSAL_MASK = None


def _is_causal(mask):
    global _CAUSAL_MASK
    m = np.asarray(mask).reshape(SQ, SQ)
    if _CAUSAL_MASK is None:
        _CAUSAL_MASK = np.triu(np.ones((SQ, SQ), dtype=bool), k=1)
    return np.array_equal(m, _CAUSAL_MASK)


def make_in_maps(inputs):
    hidden_states = np.asarray(inputs["hidden_states"], np.float32)
    qkv_w = np.asarray(inputs["qkv_w"], np.float32)
    qkv_b = np.asarray(inputs["qkv_b"], np.float32)
    svd_token = np.ascontiguousarray(np.asarray(inputs["svd_token"],
                                                np.float32))
    svd_tokenT = np.ascontiguousarray(svd_token.T)
    stok_b = svd_token.astype(np.float16)
    stokT_b = svd_tokenT.astype(np.float16)
    # fold sqrt(1/sqrt(hd)) into the qk rotation: scores come pre-scaled
    sq_scale = math.sqrt(SCALE)
    svd_qk = (np.asarray(inputs["svd_qk"], np.float32)
              * sq_scale).astype(np.float16)
    svd_vlin = np.asarray(inputs["svd_vlin"], np.float32).astype(np.float16)
    dense_w = np.asarray(inputs["dense_w"], np.float32).astype(np.float16)
    dense_b = np.asarray(inputs["dense_b"], np.float32)

    hTf = np.ascontiguousarray(hidden_states[:, 0, :].T)        # [H, SQ]
    qwTf = np.ascontiguousarray(qkv_w.T)                        # [H, 3H]
    in_maps = []
    for c in range(N_CORES):
        h0 = c * HPC
        rows = slice(c * QKVR, (c + 1) * QKVR)
        in_maps.append({
            "hT": np.ascontiguousarray(
                hTf[:, c * SEQB:(c + 1) * SEQB]).astype(np.float16),
            "qwT": np.ascontiguousarray(qwTf[:, rows]).astype(np.float16),
            "qbT": np.ascontiguousarray(qkv_b[rows].reshape(MT, 128).T),
            "stok": stok_b,
            "stokT": stokT_b,
            "sqk": np.ascontiguousarray(svd_qk[h0:h0 + HPC]),
            "svl": np.ascontiguousarray(svd_vlin[h0:h0 + HPC]),
            "dw": np.ascontiguousarray(dense_w[h0:h0 + HPC]),
            "dbB": np.ascontiguousarray(dense_b.reshape(1, H)),
        })
    return in_maps


def kernel(hidden_states, attention_mask, qkv_w, qkv_b, svd_token,
           svd_qk, svd_vlin, dense_w, dense_b):
    causal = _is_causal(attention_mask)
    if not causal:
        assert not np.asarray(attention_mask).any(), \
            "kernel supports causal or empty attention_mask"

    nc = build(causal=causal)
    in_maps = make_in_maps({
        "hidden_states": hidden_states, "qkv_w": qkv_w, "qkv_b": qkv_b,
        "svd_token": svd_token, "svd_qk": svd_qk, "svd_vlin": svd_vlin,
        "dense_w": dense_w, "dense_b": dense_b,
    })
    res = bass_utils.run_bass_kernel_spmd(
        nc, in_maps, core_ids=list(range(N_CORES)), trace=False)
    full = np.concatenate([res.results[c]["out"] for c in range(N_CORES)],
                          axis=0)
    return full.reshape(SQ, 1, H)


# revision 40
# speedup vs baseline: 1.0941x; 1.0941x over previous
"""Trainium2 8-core tensor-parallel sparse-attention kernel (Bass/Tile).

Reference (SQ=2048, B=1, H=2048, NH=16, HD=128):
    x = hidden[:,0,:] @ svd_token
    w = qkv_w @ svd_token;  mixed = x @ w.T + qkv_b
    per head h: q,k rotated by svd_qk[h], v by svd_vlin[h]
    scores = qr @ kr.T / sqrt(128) causal-masked, softmax
    ctx = probs @ vr;  tsr[h] = svd_vlin[h].T @ dense_w[h]
    out = ctx @ tsr + dense_b

Key identity: mixed = x @ (qkv_w @ st).T = (x @ st.T) @ qkv_w.T = y @ qkv_w.T
 -> compute y seq-sharded (2.15 GF/core) instead of w head-sharded (6.4).

Per-core pipeline (TP over heads, 2 heads/core):
  warmup AG first (CC stream init ~55us)
  All bulk HBM loads stream on the sync queue in consumption order
  (ring FIFO = program order): hT tiles + stok rows batched in pairs
  (pass1), stokT half-rows (pass2 per j2-half), qwT, dw.
  pass2 per j2-half -> AG_y per half triggers early, overlaps compute
  B2:    mixed^T = qkv_w-shard @ y^T, rank-pairs N=512, fused
         (+bias+half-merge) epilogue via scalar_tensor_tensor
  C:     rotations for both heads up front (own PSUM pool, PSUM->SBUF
         copies split across scalar/vector); per head, two rb-chains
         interleaved for PE density; causal mask preloaded into PSUM
         by an identity matmul (no cross-engine mask hop); raw exp on
         [128,1024] paired tiles; P@V + ones row-sum; normalize via
         partition_broadcast + reciprocal_approx_fast
  A2A:   per head -- A2A(h0) hides under h1's attention
  E:     out = ctx_myblock @ tsr + dense_b, split into h0/h1 halves so
         the h0 half of the contraction overlaps A2A(h1)
Host only shards inputs / concatenates the 8 output row-blocks.
"""
import math

import ml_dtypes
import numpy as np

import concourse.bass as bass
import concourse.mybir as mybir
import concourse.bacc as bacc
import concourse.tile as tile
from concourse import bass_utils

N_CORES = 8
SQ = 2048
H = 2048
NH = 16
HD = 128
HPC = NH // N_CORES          # heads per core = 2
QKVR = HPC * 3 * HD          # qkv rows per core = 768
SEQB = SQ // N_CORES         # seq block per core = 256
KT = H // 128                # 128-tiles over hidden = 16
MT = QKVR // 128             # qkv row tiles = 6
F32 = mybir.dt.float32
F32R = mybir.dt.float32r
BF16 = mybir.dt.bfloat16
FP16 = mybir.dt.float16
SCALE = 1.0 / math.sqrt(HD)


def r(ap):
    return ap.bitcast(F32R)


def build(causal=True):
    nc = bacc.Bacc("TRN2", target_bir_lowering=False, debug=False,
                   num_devices=N_CORES)

    hT = nc.dram_tensor("hT", [H, SEQB], FP16, kind="ExternalInput")
    qwT = nc.dram_tensor("qwT", [H, QKVR], FP16, kind="ExternalInput")
    qbT = nc.dram_tensor("qbT", [128, MT], F32, kind="ExternalInput")
    stok = nc.dram_tensor("stok", [H, H], FP16, kind="ExternalInput")
    stokT = nc.dram_tensor("stokT", [H, H], FP16, kind="ExternalInput")
    sqk = nc.dram_tensor("sqk", [HPC, HD, HD], FP16, kind="ExternalInput")
    svl = nc.dram_tensor("svl", [HPC, HD, HD], FP16, kind="ExternalInput")
    dw = nc.dram_tensor("dw", [HPC, HD, H], FP16, kind="ExternalInput")
    dbB = nc.dram_tensor("dbB", [1, H], F32, kind="ExternalInput")
    out = nc.dram_tensor("out", [SEQB, H], F32, kind="ExternalOutput")

    ones_dram = nc.inline_tensor(np.ones((128, 128), np.float32), name="ones_c")
    onesb_dram = nc.inline_tensor(np.ones((128, 128), ml_dtypes.bfloat16),
                                  name="onesb_c")
    idh_dram = nc.inline_tensor(np.eye(128, dtype=np.float16), name="idh_c")
    # additive causal mask band (-30000 above diagonal), preloaded
    # into PSUM via an identity matmul so masking never leaves the PE
    tbh_np = np.where(
        np.arange(128)[:, None] > np.arange(896)[None, :] - 384, -30000.0, 0.0
    ).astype(np.float16)
    tbh_dram = nc.inline_tensor(tbh_np, name="tbh_c")

    rg = [list(range(N_CORES))]

    with tile.TileContext(nc) as tc:
        with (
            nc.allow_low_precision(reason="f32r/bf16 for full-rate PE"),
            tc.tile_pool(name="pers", bufs=1) as pers,
            tc.tile_pool(name="dram", bufs=1, space="DRAM") as dram,
        ):
            # ---- warmup collective ASAP (CC stream init ~55us) ----
            warm_in = dram.tile([128, 128], F32)
            warm_out = dram.tile([N_CORES * 128, 128], F32,
                                 addr_space="Shared")
            nc.sync.dma_start(warm_in[:], ones_dram[:])
            nc.gpsimd.collective_compute(
                "AllGather", mybir.AluOpType.bypass, replica_groups=rg,
                ins=[warm_in[:].opt()], outs=[warm_out[:].opt()])

            # ---- persistent constants ----
            ones_sb = pers.tile([128, 128], F32)
            onesb_sb = pers.tile([128, 128], BF16)
            tbh_sb = pers.tile([128, 896], FP16)
            idb_sb = pers.tile([128, 128], FP16)
            nc.gpsimd.dma_start(idb_sb[:], idh_dram[:])
            nc.sync.dma_start(r(ones_sb[:]), r(ones_dram[:]))
            nc.gpsimd.dma_start(onesb_sb[:], onesb_dram[:])
            nc.gpsimd.dma_start(tbh_sb[:], tbh_dram[:])
            qb_sb = pers.tile([128, MT], F32)
            nc.sync.dma_start(qb_sb[:], qbT[:])
            sqk_sb = pers.tile([128, HPC * HD], FP16)
            svl_sb = pers.tile([128, HPC * HD], FP16)
            for hl in range(HPC):
                nc.gpsimd.dma_start(sqk_sb[:, hl * HD:(hl + 1) * HD], sqk[hl])
                nc.gpsimd.dma_start(svl_sb[:, hl * HD:(hl + 1) * HD], svl[hl])
            db_sb = pers.tile([1, H], F32)
            nc.sync.dma_start(r(db_sb[:]), r(dbB[:]))

            y_ins = [dram.tile([H // 2, SEQB], FP16, name=f"y_in{q}")
                     for q in range(2)]
            y_gs = [dram.tile([N_CORES * H // 2, SEQB], FP16,
                              addr_space="Shared", name=f"y_g{q}")
                    for q in range(2)]
            tsr_in = dram.tile([HPC * HD, H], FP16)
            tsr_g = dram.tile([NH * HD, H], FP16, addr_space="Shared")

            # ---- pass1/pass2 (sync queue = consumption-ordered stream) ----
            dw_sb = pers.tile([128, HPC * H], FP16)
            with (
                tc.tile_pool(name="sA", bufs=2) as sA,
                tc.tile_pool(name="pA", bufs=8, space="PSUM") as pA,
            ):
                hT_sb = sA.tile([128, KT * SEQB], FP16, tag="hTt", bufs=1)
                # pass1: x[s, j] = sum_k hT[k, s].T @ stok[k, j]
                # (hT tile k and stok row k interleaved in ring order)
                xps = [pA.tile([128, 512], F32, tag="acc", name=f"xps{i}",
                               bufs=8) for i in range(8)]
                stok_v = stok.rearrange("(m p) j -> m p j", p=128)
                for k2 in range(KT // 2):
                    nc.sync.dma_start(
                        hT_sb[:].rearrange(
                            "p (k s) -> p k s", k=KT)[:, k2 * 2:(k2 + 1) * 2],
                        hT.rearrange("(k p) s -> p k s",
                                     p=128)[:, k2 * 2:(k2 + 1) * 2])
                    srow = sA.tile([128, 2 * H], FP16, tag="srow", bufs=4)
                    nc.sync.dma_start(
                        srow[:].rearrange("p (m j) -> p m j", m=2),
                        stok_v[k2 * 2:(k2 + 1) * 2].rearrange(
                            "m p j -> p m j"))
                    for ki in range(2):
                        k = k2 * 2 + ki
                        for st in range(2):
                            for jc in range(4):
                                nc.tensor.matmul(
                                    xps[st * 4 + jc][:],
                                    hT_sb[:, k * SEQB + st * 128:
                                          k * SEQB + (st + 1) * 128],
                                    srow[:, ki * H + jc * 512:
                                         ki * H + (jc + 1) * 512],
                                    start=(k == 0), stop=(k == KT - 1))
                x_sb = sA.tile([128, 2 * H], FP16, tag="xsb", bufs=1)
                for i in range(8):
                    if i % 2:
                        nc.vector.tensor_copy(
                            x_sb[:, i * 512:(i + 1) * 512], xps[i][:])
                    else:
                        nc.scalar.copy(
                            x_sb[:, i * 512:(i + 1) * 512], xps[i][:])
                # transpose x -> xT (m on partitions)
                xT_sb = sA.tile([128, KT * SEQB], FP16, tag="xTt", bufs=1)
                for m in range(KT):
                    tp2 = pA.tile([128, 256], FP16, tag="acc",
                                  name=f"tpx{m}", bufs=8)
                    for st in range(2):
                        nc.tensor.transpose(
                            tp2[:, st * 128:(st + 1) * 128],
                            x_sb[:, st * H + m * 128: st * H + (m + 1) * 128],
                            idb_sb[:])
                    if m % 2:
                        nc.vector.tensor_copy(
                            xT_sb[:, m * SEQB:(m + 1) * SEQB], tp2[:])
                    else:
                        nc.scalar.copy(
                            xT_sb[:, m * SEQB:(m + 1) * SEQB], tp2[:])
                # pass2 per j2-half: y[s, j2] = sum_m xT[m, s].T @ stokT[m, j2]
                y_sb = sA.tile([128, 2 * H], FP16, tag="xsb", bufs=1,
                               name="y_sb")
                for hf in range(2):
                    yps = [pA.tile([128, 512], F32, tag="acc",
                                   name=f"yps{hf}_{i}", bufs=8)
                           for i in range(4)]
                    stokT_v = stokT.rearrange("(m p) j -> m p j", p=128)
                    for m2 in range(KT // 2):
                        srow2 = sA.tile([128, 2048], FP16, tag="srow2",
                                        bufs=4)
                        nc.sync.dma_start(
                            srow2[:].rearrange("p (m j) -> p m j", m=2),
                            stokT_v[m2 * 2:(m2 + 1) * 2, :,
                                    hf * 1024:(hf + 1) * 1024]
                            .rearrange("m p j -> p m j"))
                        for mi in range(2):
                            m = m2 * 2 + mi
                            for st in range(2):
                                for jc in range(2):
                                    nc.tensor.matmul(
                                        yps[st * 2 + jc][:],
                                        xT_sb[:, m * SEQB + st * 128:
                                              m * SEQB + (st + 1) * 128],
                                        srow2[:, mi * 1024 + jc * 512:
                                              mi * 1024 + (jc + 1) * 512],
                                        start=(m == 0), stop=(m == KT - 1))
                    for st in range(2):
                        for jc in range(2):
                            if (st + jc) % 2:
                                nc.vector.tensor_copy(
                                    y_sb[:, st * H + hf * 1024 + jc * 512:
                                         st * H + hf * 1024 + (jc + 1) * 512],
                                    yps[st * 2 + jc][:])
                            else:
                                nc.scalar.copy(
                                    y_sb[:, st * H + hf * 1024 + jc * 512:
                                         st * H + hf * 1024 + (jc + 1) * 512],
                                    yps[st * 2 + jc][:])
                    # transpose this half's j2 tiles, stage 4-at-a-time, AG
                    for jq4 in range(2):
                        ystg = sA.tile([128, 4 * SEQB], FP16, tag="ystg",
                                       bufs=2)
                        for jj in range(4):
                            j2 = hf * 8 + jq4 * 4 + jj
                            tp3 = pA.tile([128, 256], FP16, tag="acc",
                                          name=f"tpy{j2}", bufs=8)
                            for st in range(2):
                                nc.tensor.transpose(
                                    tp3[:, st * 128:(st + 1) * 128],
                                    y_sb[:, st * H + j2 * 128:
                                         st * H + (j2 + 1) * 128],
                                    idb_sb[:])
                            if jj % 2:
                                nc.vector.tensor_copy(
                                    ystg[:, jj * SEQB:(jj + 1) * SEQB],
                                    tp3[:])
                            else:
                                nc.scalar.copy(
                                    ystg[:, jj * SEQB:(jj + 1) * SEQB],
                                    tp3[:])
                        nc.sync.dma_start(
                            y_ins[hf].rearrange(
                                "(j p) s -> p j s", p=128)[:, jq4 * 4:
                                                           (jq4 + 1) * 4],
                            ystg[:].rearrange("p (j s) -> p j s", j=4))
                    nc.gpsimd.collective_compute(
                        "AllGather", mybir.AluOpType.bypass,
                        replica_groups=rg,
                        ins=[y_ins[hf][:].opt()],
                        outs=[y_gs[hf][:].opt()])
                # bulk loads for later stages, ring-ordered after streams
                qwT_sb = pers.tile([128, KT * QKVR], FP16)
                for k in range(KT):
                    nc.sync.dma_start(
                        qwT_sb[:, k * QKVR:(k + 1) * QKVR],
                        qwT[k * 128:(k + 1) * 128, :])
                for hl in range(HPC):
                    nc.sync.dma_start(dw_sb[:, hl * H:(hl + 1) * H], dw[hl])

            # ---- tsr shard + AG_tsr + dense_b broadcast ----
            bb_sb = pers.tile([128, H], FP16)
            with (
                tc.tile_pool(name="s0", bufs=2) as s0,
                tc.tile_pool(name="p0", bufs=2, space="PSUM") as p0,
            ):
                for hl in range(HPC):
                    tsr_sb = s0.tile([128, H], FP16, tag="tsr")
                    for n in range(4):
                        tp = p0.tile([128, 512], F32, tag="t0p")
                        nc.tensor.matmul(
                            tp[:], svl_sb[:, hl * HD:(hl + 1) * HD],
                            dw_sb[:, hl * H + n * 512: hl * H + (n + 1) * 512],
                            start=True, stop=True)
                        nc.vector.tensor_copy(
                            tsr_sb[:, n * 512:(n + 1) * 512], tp[:])
                    nc.sync.dma_start(tsr_in[hl * HD:(hl + 1) * HD, :],
                                      tsr_sb[:])
                for n in range(4):
                    bp = p0.tile([128, 512], F32, tag="t0p")
                    nc.tensor.matmul(bp[:], r(ones_sb[0:1, :]),
                                     r(db_sb[:, n * 512:(n + 1) * 512]),
                                     start=True, stop=True)
                    nc.vector.tensor_copy(bb_sb[:, n * 512:(n + 1) * 512],
                                          bp[:])
            nc.gpsimd.collective_compute(
                "AllGather", mybir.AluOpType.bypass, replica_groups=rg,
                ins=[tsr_in[:].opt()], outs=[tsr_g[:].opt()])

            # ---- B2: mixed^T = qw_shard @ y^T, rank-pairs (N=512) ----
            mid = tc.alloc_tile_pool(name="mid", bufs=1)
            mixT = mid.tile([128, MT * SQ], FP16, name="mixT")
            tsrb_sb = mid.tile([128, KT * H], FP16, name="tsrb_sb")
            with (
                tc.tile_pool(name="sB", bufs=2) as sB,
                tc.tile_pool(name="pB", bufs=6, space="PSUM") as pB,
            ):
                half1 = sB.tile([128, 4 * MT * 512], FP16, tag="half1",
                                bufs=1)
                ygv = [y_gs[q].rearrange("(r k p) s -> r k p s",
                                         r=N_CORES, p=128) for q in range(2)]
                for rp in range(N_CORES // 2):
                    mps = [pB.tile([128, 512], F32, tag="mp",
                                   name=f"mpa{rp}_{i}") for i in range(MT)]
                    for kb in range(2):
                        yg_sb = sB.tile([128, 4 * 512], FP16, tag="yg",
                                        bufs=4)
                        for h in range(2):
                            nc.sync.dma_start(
                                yg_sb[:].rearrange("p (k h s) -> h p k s",
                                                   k=4, h=2)[h],
                                ygv[0][rp * 2 + h, kb * 4:(kb + 1) * 4]
                                .rearrange("k p s -> p k s"))
                        for k4 in range(4):
                            k = kb * 4 + k4
                            for mt in range(MT):
                                nc.tensor.matmul(
                                    mps[mt][:],
                                    qwT_sb[:, k * QKVR + mt * 128:
                                           k * QKVR + (mt + 1) * 128],
                                    yg_sb[:, k4 * 512:(k4 + 1) * 512],
                                    start=(k == 0), stop=(k == KT // 2 - 1))
                    for mt in range(MT):
                        nc.vector.tensor_copy(
                            half1[:, (rp * MT + mt) * 512:
                                  (rp * MT + mt + 1) * 512], mps[mt][:])
                for rp in range(N_CORES // 2):
                    mps2 = [pB.tile([128, 512], F32, tag="mp",
                                    name=f"mpb{rp}_{i}") for i in range(MT)]
                    for kb in range(2):
                        yg_sb = sB.tile([128, 4 * 512], FP16, tag="yg",
                                        bufs=4)
                        for h in range(2):
                            nc.sync.dma_start(
                                yg_sb[:].rearrange("p (k h s) -> h p k s",
                                                   k=4, h=2)[h],
                                ygv[1][rp * 2 + h, kb * 4:(kb + 1) * 4]
                                .rearrange("k p s -> p k s"))
                        for k4 in range(4):
                            k = KT // 2 + kb * 4 + k4
                            for mt in range(MT):
                                nc.tensor.matmul(
                                    mps2[mt][:],
                                    qwT_sb[:, k * QKVR + mt * 128:
                                           k * QKVR + (mt + 1) * 128],
                                    yg_sb[:, k4 * 512:(k4 + 1) * 512],
                                    start=(k == KT // 2), stop=(k == KT - 1))
                    for mt in range(MT):
                        nc.vector.scalar_tensor_tensor(
                            mixT[:, mt * SQ + rp * 512:
                                 mt * SQ + (rp + 1) * 512],
                            mps2[mt][:], qb_sb[:, mt:mt + 1],
                            half1[:, (rp * MT + mt) * 512:
                                  (rp * MT + mt + 1) * 512],
                            mybir.AluOpType.add, mybir.AluOpType.add)

            # ---- stage C: rotations + attention per head ----
            ctx_in = dram.tile([N_CORES, HPC * HD, SEQB], FP16, name="ctxin")
            ctx_a = dram.tile([N_CORES, HPC * HD, SEQB], FP16, name="ctxa")
            # prefetch tsr during stage C on the idle gpsimd queue
            for kt in range(KT):
                nc.gpsimd.dma_start(
                    tsrb_sb[:, kt * H:(kt + 1) * H],
                    tsr_g[kt * 128:(kt + 1) * 128, :])
            with (
                tc.tile_pool(name="sC", bufs=1) as sC,
                tc.tile_pool(name="sD", bufs=2) as sD,
            ):
                qrotTs, krotTs, vrots, ctxTs = [], [], [], []
                with tc.tile_pool(name="pR", bufs=2, space="PSUM") as pR:
                    for hl in range(HPC):
                        qrow, krow, vrow = hl * 3, hl * 3 + 1, hl * 3 + 2
                        qrotT = sC.tile([128, SQ], FP16, tag="qrot", bufs=2,
                                        name=f"qrotT{hl}")
                        krotT = sC.tile([128, SQ], FP16, tag="krot", bufs=2,
                                        name=f"krotT{hl}")
                        vrot = sC.tile([128, SQ], BF16, tag="vrot", bufs=2,
                                       name=f"vrot{hl}")
                        for sc in range(4):
                            rp1 = pR.tile([128, 512], F32, tag="rotp")
                            nc.tensor.matmul(
                                rp1[:], sqk_sb[:, hl * HD:(hl + 1) * HD],
                                mixT[:, qrow * SQ + sc * 512:
                                     qrow * SQ + (sc + 1) * 512],
                                start=True, stop=True)
                            nc.scalar.copy(
                                qrotT[:, sc * 512:(sc + 1) * 512], rp1[:])
                            rp2 = pR.tile([128, 512], F32, tag="rotp")
                            nc.tensor.matmul(
                                rp2[:], sqk_sb[:, hl * HD:(hl + 1) * HD],
                                mixT[:, krow * SQ + sc * 512:
                                     krow * SQ + (sc + 1) * 512],
                                start=True, stop=True)
                            nc.vector.tensor_copy(
                                krotT[:, sc * 512:(sc + 1) * 512], rp2[:])
                        for st in range(KT):
                            vp = pR.tile([128, 128], F32, tag="rotp")
                            nc.tensor.matmul(
                                vp[:],
                                mixT[:, vrow * SQ + st * 128:
                                     vrow * SQ + (st + 1) * 128],
                                svl_sb[:, hl * HD:(hl + 1) * HD],
                                start=True, stop=True)
                            if st % 2:
                                nc.vector.tensor_copy(
                                    vrot[:, st * 128:(st + 1) * 128], vp[:])
                            else:
                                nc.scalar.copy(
                                    vrot[:, st * 128:(st + 1) * 128], vp[:])
                        ctxT_sb = sC.tile([128, SQ], FP16, tag="ctxT",
                                          bufs=2, name=f"ctxT{hl}")
                        qrotTs.append(qrotT); krotTs.append(krotT)
                        vrots.append(vrot); ctxTs.append(ctxT_sb)

                with tc.tile_pool(name="pC", bufs=2, space="PSUM") as pC:
                    for rb in range(4):
                        ncb = 4 * (rb + 1) if causal else KT
                        ctps = [pC.tile([128, 512], F32, tag="ctp",
                                        name=f"ctp{hl}_{rb}")
                                for hl in range(HPC)]
                        lps = [pC.tile([1, 512], F32, tag="lp", bufs=2,
                                       name=f"lp{hl}_{rb}")
                               for hl in range(HPC)]
                        # interleave the two heads: while one head's chain
                        # waits on exp, the other's matmuls fill the PE
                        for cp in range(ncb // 2):
                            for hl in range(HPC):
                                qrotT, krotT = qrotTs[hl], krotTs[hl]
                                vrot = vrots[hl]
                                sp = pC.tile([128, 1024], F32, tag="sp",
                                             name=f"sp{hl}_{rb}_{cp}")
                                pT = sD.tile([128, 1024], BF16, tag="pT",
                                             bufs=6,
                                             name=f"pT{hl}_{rb}_{cp}")
                                for ch in range(2):
                                    cb = cp * 2 + ch
                                    sph = sp[:, ch * 512:(ch + 1) * 512]
                                    masked = causal and cb >= 4 * rb
                                    if masked:
                                        o = 384 - (cb * 128 - rb * 512)
                                        nc.tensor.matmul(
                                            sph, idb_sb[:],
                                            tbh_sb[:, o:o + 512],
                                            start=True, stop=False)
                                    nc.tensor.matmul(
                                        sph,
                                        krotT[:, cb * 128:(cb + 1) * 128],
                                        qrotT[:, rb * 512:(rb + 1) * 512],
                                        start=not masked, stop=True)
                                nc.scalar.activation(
                                    pT[:], sp[:],
                                    mybir.ActivationFunctionType.Exp)
                                for ch in range(2):
                                    cb = cp * 2 + ch
                                    pTh = pT[:, ch * 512:(ch + 1) * 512]
                                    nc.tensor.matmul(
                                        ctps[hl][:],
                                        vrot[:, cb * 128:(cb + 1) * 128],
                                        pTh,
                                        start=(cb == 0),
                                        stop=(cb == ncb - 1))
                                    nc.tensor.matmul(
                                        lps[hl][:], onesb_sb[:, 0:1], pTh,
                                        start=(cb == 0),
                                        stop=(cb == ncb - 1))
                        for hl in range(HPC):
                            ctxT_sb = ctxTs[hl]
                            # normalize: lp -> SBUF, partition-broadcast,
                            # fast reciprocal (128 lanes), mult
                            lsb = sD.tile([1, 512], F32, tag="lsb",
                                          name=f"lsb{hl}_{rb}")
                            nc.scalar.copy(lsb[:], lps[hl][:])
                            lball = sD.tile([128, 512], F32, tag="lball",
                                            name=f"lball{hl}_{rb}")
                            nc.gpsimd.partition_broadcast(lball[:], lsb[:])
                            linvb = sD.tile([128, 512], F32, tag="lb",
                                            name=f"linvb{hl}_{rb}")
                            nc.vector.reciprocal_approx_fast(linvb[:],
                                                             lball[:])
                            nc.vector.tensor_tensor(
                                ctxT_sb[:, rb * 512:(rb + 1) * 512],
                                ctps[hl][:],
                                linvb[:], mybir.AluOpType.mult)
                            # stage ctx columns (dest cores 2rb, 2rb+1)
                            for half in range(2):
                                b = 2 * rb + half
                                nc.sync.dma_start(
                                    ctx_in[b, hl * HD:(hl + 1) * HD, :],
                                    ctxT_sb[:, b * SEQB:(b + 1) * SEQB])
                nc.gpsimd.collective_compute(
                    "AllToAll", mybir.AluOpType.bypass, replica_groups=rg,
                    ins=[ctx_in[:].opt()], outs=[ctx_a[:].opt()])

            # ---- stage E: out = ctx_myblock @ tsr + dense_b ----
            with (
                tc.tile_pool(name="sE", bufs=2) as sE,
                tc.tile_pool(name="pE", bufs=4, space="PSUM") as pE,
            ):
                ctxa_sb = sE.tile([128, KT * SEQB], FP16, tag="ctxa", bufs=1)
                for b in range(N_CORES):
                    nc.sync.dma_start(
                        ctxa_sb[:].rearrange("p (b h s) -> b p h s",
                                             b=N_CORES, h=HPC)[b],
                        ctx_a[b].rearrange("(h p) s -> p h s", p=128))
                for mt in range(2):
                    ops = [pE.tile([128, 512], F32, tag="op",
                                   name=f"op{mt}_{n}") for n in range(4)]
                    for kt in range(KT):
                        for n in range(4):
                            nc.tensor.matmul(
                                ops[n][:],
                                ctxa_sb[:, kt * SEQB + mt * 128:
                                        kt * SEQB + (mt + 1) * 128],
                                tsrb_sb[:, kt * H + n * 512:
                                        kt * H + (n + 1) * 512],
                                start=(kt == 0), stop=(kt == KT - 1))
                    for n in range(4):
                        os_ = sE.tile([128, 512], F32, tag="os")
                        nc.vector.tensor_tensor(
                            os_[:], ops[n][:],
                            bb_sb[:, n * 512:(n + 1) * 512],
                            mybir.AluOpType.add)
                        nc.sync.dma_start(
                            out[mt * 128:(mt + 1) * 128,
                                n * 512:(n + 1) * 512],
                            os_[:])
            mid.release()
    nc.compile()
    return nc


_CAUSAL_MASK = None


def _is_causal(mask):
    global _CAUSAL_MASK
    m = np.asarray(mask).reshape(SQ, SQ)
    if _CAUSAL_MASK is None:
        _CAUSAL_MASK = np.triu(np.ones((SQ, SQ), dtype=bool), k=1)
    return np.array_equal(m, _CAUSAL_MASK)


def make_in_maps(inputs):
    hidden_states = np.asarray(inputs["hidden_states"], np.float32)
    qkv_w = np.asarray(inputs["qkv_w"], np.float32)
    qkv_b = np.asarray(inputs["qkv_b"], np.float32)
    svd_token = np.ascontiguousarray(np.asarray(inputs["svd_token"],
                                                np.float32))
    svd_tokenT = np.ascontiguousarray(svd_token.T)
    stok_b = svd_token.astype(np.float16)
    stokT_b = svd_tokenT.astype(np.float16)
    # fold sqrt(1/sqrt(hd)) into the qk rotation: scores come pre-scaled
    sq_scale = math.sqrt(SCALE)
    svd_qk = (np.asarray(inputs["svd_qk"], np.float32)
              * sq_scale).astype(np.float16)
    svd_vlin = np.asarray(inputs["svd_vlin"], np.float32).astype(np.float16)
    dense_w = np.asarray(inputs["dense_w"], np.float32).astype(np.float16)
    dense_b = np.asarray(inputs["dense_b"], np.float32)

    hTf = np.ascontiguousarray(hidden_states[:, 0, :].T)        # [H, SQ]
    qwTf = np.ascontiguousarray(qkv_w.T)                        # [H, 3H]
    in_maps = []
    for c in range(N_CORES):
        h0 = c * HPC
        rows = slice(c * QKVR, (c + 1) * QKVR)
        in_maps.append({
            "hT": np.ascontiguousarray(
                hTf[:, c * SEQB:(c + 1) * SEQB]).astype(np.float16),
            "qwT": np.ascontiguousarray(qwTf[:, rows]).astype(np.float16),
            "qbT": np.ascontiguousarray(qkv_b[rows].reshape(MT, 128).T),
            "stok": stok_b,
            "stokT": stokT_b,
            "sqk": np.ascontiguousarray(svd_qk[h0:h0 + HPC]),
            "svl": np.ascontiguousarray(svd_vlin[h0:h0 + HPC]),
            "dw": np.ascontiguousarray(dense_w[h0:h0 + HPC]),
            "dbB": np.ascontiguousarray(dense_b.reshape(1, H)),
        })
    return in_maps


def kernel(hidden_states, attention_mask, qkv_w, qkv_b, svd_token,
           svd_qk, svd_vlin, dense_w, dense_b):
    causal = _is_causal(attention_mask)
    if not causal:
        assert not np.asarray(attention_mask).any(), \
            "kernel supports causal or empty attention_mask"

    nc = build(causal=causal)
    in_maps = make_in_maps({
        "hidden_states": hidden_states, "qkv_w": qkv_w, "qkv_b": qkv_b,
        "svd_token": svd_token, "svd_qk": svd_qk, "svd_vlin": svd_vlin,
        "dense_w": dense_w, "dense_b": dense_b,
    })
    res = bass_utils.run_bass_kernel_spmd(
        nc, in_maps, core_ids=list(range(N_CORES)), trace=False)
    full = np.concatenate([res.results[c]["out"] for c in range(N_CORES)],
                          axis=0)
    return full.reshape(SQ, 1, H)
